# revision 37
# baseline (speedup 1.0000x reference)
"""C3D loss kernel for Trainium2 (8 NeuronCores, Bass/Tile).

Sharding: pure data parallel over B*2 = 8 shards (each image split into
top/bottom 176-row halves). Each core computes a partial sum of the loss
numerator; host combines and divides by the valid count.

Layout: partitions = 122 column blocks of 10 pixels (3+3 col halo -> 16
stored cols per block); free dims = (rows, 16). Every spatial shift (the
5x5 window and the normal central differences) is a free-dim offset, which
keeps all engine accesses at partition start 0 (a hardware requirement).

v3 design:
- Inputs staged on the host directly in block layout [NT, NB, rows, 16],
  so each load DMA moves contiguous 6KB runs per partition.
- The host folds the SQS prescale and the xy1 z-channel into the depth
  planes (xyz_z == scaled depth), so only x/y need on-device muls.
- No channel stacking: window diffs / squares / normal products run
  per-channel on [122, 880] views, and the channel sum d2 = sum_c diff_c^2
  rides the SAME identity-stationary PSUM-accumulating matmuls that the
  offset accumulation already uses. This removes 36 SBUF->SBUF stacking
  DMAs per tile.
- A custom DVE op SQDIFF_C3D computes (a-b)^2 in one f32-in instruction;
  a policy splits offsets between it and Pool-sub+Act-square to balance
  engines.
- |nd| rides Act as Abs(1.9*nd); the +0.1 coefficient is a second
  accumulation matmul with 0.1*I stationary.
- The normal chain stays f32 through the cross product (gx ~parallel~ gy
  amplifies rounding ~30x); gt normals only computed on the 88x10 output
  domain.
- exp/abs/trm for offset o-1 are emitted at the head of turn o and the
  accumulation matmuls one turn later, so Act/DVE/PE pipeline across
  offsets.
"""
import sys

sys.path.insert(0, "/opt/trn_rl_repo")

import numpy as np
from contextlib import ExitStack

import bass_rust
import concourse.bass as bass
import concourse.tile as tile
import concourse.dve_ops as dve_ops
import concourse.dve_spec as dve_spec
from concourse.dve_spec import Spec, Src0, Src1, sq
from concourse.dve_uop import DveOpSpec
from concourse import bacc, mybir
from concourse.bass_utils import run_bass_kernel_spmd

F32 = mybir.dt.float32
F16 = mybir.dt.float16
AF = mybir.ActivationFunctionType
ALU = mybir.AluOpType

B, H, W = 4, 352, 1216
R = 2
ELL = 0.05
INV2ELL2 = float(np.float32(1.0 / (2.0 * ELL * ELL)))   # 200.0
EPS = 1e-8
N_CORES = 8

SH = H // 2          # shard rows per core = 176
NT = 2               # row tiles per core
TR = SH // NT        # output rows per tile = 88
HH = TR // 2         # PSUM chunk rows = 44
RB = TR + 6          # stored rows per tile = 94
CB = 10              # cols per block
NB = 122             # blocks
BW = CB + 6          # stored cols per block = 16
SW = CB * (NB - 1) + BW   # slab width = 1226 (slab col j <-> image col j-3)
SQS = 0.0625         # pre-scale (2^-4, exact) folded into inputs on host
PZ = 2000.0 * SQS    # poison depth in scaled units = 125
EXS = float(INV2ELL2 / (SQS * SQS))    # exp scale compensation = 51200
LN14 = float(np.log(0.25))

# engine policy for the per-offset squared diffs, indexed by
# (t*25+oi)*3+c mod len: 'a' = fused SQDIFF on DVE,
# 'b' = sub on Pool + square on Act, 'c' = sub on Pool + square on DVE
POL_SBSQ = ('a', 'b', 'b', 'a', 'b', 'b', 'a', 'b', 'b', 'a', 'b', 'b', 'a', 'a', 'b')
NPR_POOL = 9999  # every NPR_POOL-th npr mul runs on Pool instead of DVE
NLD = 4       # DMA chunks per input plane load
_prog_cache = {}


def _register_sqdiff():
    name = "SQDIFF_C3D"
    if name in dve_ops._SUB_OPCODE_FOR_NAME:
        for o in dve_ops.OPS:
            if o.name == name:
                return o
    spec = Spec(
        body=sq(Src0 - Src1),
        reference=lambda in0, in1, s0, s1, imm2:
            ((in0.astype(np.float32) - in1) ** 2).astype(np.float32))
    row = max(dve_ops._SUB_OPCODE_FOR_NAME.values()) + 1
    assert row < 0x20
    dve_ops._SUB_OPCODE_FOR_NAME[name] = row
    shas = {}
    for ver in ("v3", "v4"):
        uops = dve_spec.lower(spec, ver=ver)
        s = DveOpSpec(name=name, opcode=row, uops=uops,
                      rd1_en=dve_spec._has_src1(spec))
        shas[ver] = s.sha(ver)
    op = dve_ops.DveOp(name, spec, subdim=False, uops_sha=shas)
    dve_ops.OPS.append(op)
    dve_ops.CUSTOM_DVE_SPECS[name] = spec
    return op


def _build_program():
    sqdiff = _register_sqdiff()
    nc = bacc.Bacc("TRN2", target_bir_lowering=False, debug=False,
                   num_devices=N_CORES)

    for v in (EPS, LN14, 1e-30):
        t = nc.alloc_sbuf_tensor(f"const-f32-{v}", [128, 1], F32)
        nc.gpsimd.memset(t.ap(), v)
        nc.const_aps.aps[(F32, v)] = t.ap()
    nc.all_engine_barrier()

    dp_d = nc.dram_tensor("dp", [NT, NB, RB, BW], F32, kind="ExternalInput").ap()
    dg_d = nc.dram_tensor("dg", [NT, NB, RB, BW], F32, kind="ExternalInput").ap()
    xy1_d = nc.dram_tensor("xy1", [2, NT, NB, RB, BW], F32,
                           kind="ExternalInput").ap()
    mk_d = nc.dram_tensor("mk", [NT, NB, TR, CB], F32, kind="ExternalInput").ap()
    st_d = nc.dram_tensor("strip", [2, 3, NB, 3, BW], F32,
                          kind="ExternalInput").ap()
    pz_d = nc.dram_tensor("pzc", [3, RB, 2], F32, kind="ExternalInput").ap()
    id_d = nc.dram_tensor("idm", [NB, NB], F16, kind="ExternalInput").ap()
    id01_d = nc.dram_tensor("idm01", [NB, NB], F16, kind="ExternalInput").ap()
    out_d = nc.dram_tensor("out", [128, NT], F32, kind="ExternalOutput").ap()

    def chunked_load(dst, src):
        step = (NB + NLD - 1) // NLD
        for i0 in range(0, NB, step):
            i1 = min(i0 + step, NB)
            nc.sync.dma_start(out=dst[i0:i1], in_=src[i0:i1])

    with tile.TileContext(nc) as tc, ExitStack() as ctx:
        pool = ctx.enter_context(tc.tile_pool(name="p", bufs=1))
        psum = ctx.enter_context(tc.tile_pool(name="ps", bufs=1, space="PSUM"))
        idt = pool.tile([NB, NB], F16, name="idt")
        nc.sync.dma_start(out=idt[:], in_=id_d[:])
        idt01 = pool.tile([NB, NB], F16, name="idt01")
        nc.sync.dma_start(out=idt01[:], in_=id01_d[:])

        for t in range(NT):
            # ---------------- input loads (block layout, contiguous) -------
            dpt = pool.tile([NB, RB, BW], F32, name="dpt", bufs=2)
            chunked_load(dpt, dp_d[t])
            dgt = pool.tile([NB, RB, BW], F32, name="dgt", bufs=2)
            chunked_load(dgt, dg_d[t])
            xy1t = [pool.tile([NB, RB, BW], F32, name=f"xy1t{c}",
                              bufs=(2 if c == 0 else 1))
                    for c in range(2)]
            for c in range(2):
                chunked_load(xy1t[c], xy1_d[c, t])
            mkt = pool.tile([NB, TR, CB], F32, name="mkt")
            nc.sync.dma_start(out=mkt[:], in_=mk_d[t])

            # ------- xyz: z plane IS the (scaled) depth plane -------
            xp = [pool.tile([NB, RB, BW], F32, name=f"xp{c}", bufs=2)
                  for c in range(2)]
            xg = [pool.tile([NB, RB, BW], F32, name=f"xg{c}", bufs=2)
                  for c in range(2)]
            for c in range(2):
                nc.vector.tensor_mul(xp[c][:], xy1t[c][:], dpt[:])
                nc.gpsimd.tensor_mul(xg[c][:, 2:92, 2:14], xy1t[c][:, 2:92, 2:14],
                                     dgt[:, 2:92, 2:14])
            xp.append(dpt)
            xg.append(dgt)

            # ---------------- normals (f32 chain) ----------------
            def w3(x, dr, dc):
                return x[:, 1 + dr:93 + dr, 1 + dc:15 + dc]

            nrm = {}
            for key, xc in (("p", xp), ("g", xg)):
                seng = nc.vector if key == "p" else nc.gpsimd
                gx = [pool.tile([NB, 92, 14], F32, name=f"gx{c}")
                      for c in range(3)]
                gy = [pool.tile([NB, 92, 14], F32, name=f"gy{c}")
                      for c in range(3)]
                if key == "p":
                    sub = lambda x: x[:, 0:92, 0:14]
                    vx0, vx1 = (lambda c: w3(xc[c], 0, 1)), (lambda c: w3(xc[c], 0, -1))
                    vy0, vy1 = (lambda c: w3(xc[c], 1, 0)), (lambda c: w3(xc[c], -1, 0))
                else:
                    sub = lambda x: x[:, 0:TR, 0:CB]
                    vx0 = lambda c: xc[c][:, 3:91, 4:14]
                    vx1 = lambda c: xc[c][:, 3:91, 2:12]
                    vy0 = lambda c: xc[c][:, 4:92, 3:13]
                    vy1 = lambda c: xc[c][:, 2:90, 3:13]
                for c in range(3):
                    seng.tensor_sub(sub(gx[c]), vx0(c), vx1(c))
                    nc.vector.tensor_sub(sub(gy[c]), vy0(c), vy1(c))
                cr = [pool.tile([NB, 92, 14], F32, name=f"cr{c}")
                      for c in range(3)]
                tA = pool.tile([NB, 92, 14], F32, name="tA")
                for c in range(3):
                    a, b = (c + 1) % 3, (c + 2) % 3
                    nc.vector.tensor_mul(sub(cr[c]), sub(gx[a]), sub(gy[b]))
                    seng.tensor_mul(sub(tA), sub(gx[b]), sub(gy[a]))
                    seng.tensor_sub(sub(cr[c]), sub(cr[c]), sub(tA))
                q = pool.tile([NB, 92, 14], F32, name="q")
                sqt = pool.tile([NB, 92, 14], F32, name="sqt", tag="tA")
                nc.scalar.activation(sub(q), sub(cr[0]), AF.Square)
                nc.scalar.activation(sub(sqt), sub(cr[1]), AF.Square)
                seng.tensor_add(sub(q), sub(q), sub(sqt))
                nc.scalar.activation(sub(sqt), sub(cr[2]), AF.Square)
                seng.tensor_add(sub(q), sub(q), sub(sqt))
                # w = q^-0.5 = exp(-0.5*ln(q+1e-30)); exact-zero borders
                # give w=1e15 * cr=0 -> n=0, matching n/(|n|+eps)
                nc.scalar.activation(sub(q), sub(q), AF.Ln, bias=1e-30)
                nc.scalar.activation(sub(q), sub(q), AF.Exp, scale=-0.5)
                nshape = [NB, 3, 92, 14] if key == "p" else [NB, 3, TR, CB]
                nt_ = pool.tile(nshape, F16, name=f"n{key}A", bufs=2)
                for c in range(3):
                    nc.vector.tensor_mul(sub(nt_[:, c]), sub(cr[c]), sub(q))
                nrm[key] = nt_
            npn, ngn = nrm["p"], nrm["g"]

            # ------- poison xp at out-of-image rows/cols (after normals) ---
            for c in range(3):
                if t == 0:
                    nc.sync.dma_start(out=xp[c][:, 1:3, 1:15],
                                      in_=st_d[0, c, :, 1:3, 1:15])
                if t == NT - 1:
                    nc.sync.dma_start(out=xp[c][:, 91:93, 1:15],
                                      in_=st_d[1, c, :, 0:2, 1:15])
                nc.sync.dma_start(out=xp[c][0:1, 1:93, 1:3],
                                  in_=pz_d[c, 0:92, :])
                nc.sync.dma_start(out=xp[c][NB - 1:NB, 1:93, 9:11],
                                  in_=pz_d[c, 0:92, :])

            # ---------------- window phase ----------------
            accP = [psum.tile([NB, HH, CB], F32, name=f"accP{ch}") for ch in range(2)]
            ndP = psum.tile([NB, 2, 512], F32, name="ndP")

            noff = (2 * R + 1) ** 2
            offs = [(dy, dx) for dy in range(-R, R + 1) for dx in range(-R, R + 1)]

            def acc_trm(ptrm, poi):
                for ch in range(2):
                    rs = slice(ch * HH, (ch + 1) * HH)
                    nc.tensor.matmul(accP[ch][:], idt[:], ptrm[:, rs, :],
                                     start=(poi == 0), stop=False)

            def acc_kgt(pkgt, poi):
                for ch in range(2):
                    rs = slice(ch * HH, (ch + 1) * HH)
                    nc.tensor.matmul(accP[ch][:], idt01[:], pkgt[:, rs, :],
                                     start=False, stop=(poi == noff - 1))

            def finish(pend):
                # exp/abs/trm for the PREVIOUS offset: at the head of the
                # next turn so Act/DVE don't lock-step with the PE.
                d2p, kgt, stt, trm, poi = pend
                if d2p is not None:
                    nc.scalar.activation(
                        kgt[:].rearrange("p (a r) c -> p a (r c)", a=2),
                        d2p[:, :, 0:HH * CB], AF.Exp, scale=-EXS)
                nc.scalar.activation(
                    stt[:].rearrange("p (a r) c -> p a (r c)", a=2),
                    ndP[:, :, 0:HH * CB], AF.Abs, scale=1.9)
                nc.vector.tensor_mul(trm[:], stt[:], kgt[:])

            def emit_d2(oi, dy, dx, d2P, kgt, sbq):
                for c in range(3):
                    xsv = xp[c][:, 3 + dy:91 + dy, 3 + dx:13 + dx]
                    xgv = xg[c][:, 3:91, 3:13]
                    gi = (t * noff + oi) * 3 + c
                    var = POL_SBSQ[gi % len(POL_SBSQ)]
                    if var == 'a':
                        nc.vector._custom_dve(sqdiff, out=sbq[c][:],
                                              in0=xsv, in1=xgv)
                    else:
                        nc.gpsimd.tensor_sub(sbq[c][:], xsv, xgv)
                        if var == 'b':
                            nc.scalar.activation(sbq[c][:], sbq[c][:],
                                                 AF.Square)
                        else:
                            nc.vector.tensor_mul(sbq[c][:], sbq[c][:],
                                                 sbq[c][:])
                for c in range(3):
                    for ch in range(2):
                        rs = slice(ch * HH, (ch + 1) * HH)
                        nc.tensor.matmul(d2P[:, ch, 0:HH * CB]
                                         .rearrange("p (r c) -> p r c", c=CB),
                                         idt[:], sbq[c][:, rs, :],
                                         start=(c == 0), stop=(c == 2))

            # head start: the d2 side of the first K offsets (+ eager exp)
            # only needs xp/xg/poison, so it runs while the normal chain
            # finishes; the nd side catches up in the main loop.
            K = 2 if t == 0 else 0
            heads = {}
            for oi in range(K):
                dy, dx = offs[oi]
                d2P = psum.tile([NB, 2, 512], F32, name="d2P", tag="d2P",
                                bufs=2)
                kgt = pool.tile([NB, TR, CB], F16, name="kgt", tag="kgt",
                                bufs=3)
                sbq = [pool.tile([NB, TR, CB], F16, name=f"sbq{c}",
                                 tag=f"sbq{c}", bufs=2) for c in range(3)]
                emit_d2(oi, dy, dx, d2P, kgt, sbq)
                nc.scalar.activation(
                    kgt[:].rearrange("p (a r) c -> p a (r c)", a=2),
                    d2P[:, :, 0:HH * CB], AF.Exp, scale=-EXS)
                heads[oi] = kgt

            pend = None
            for oi, (dy, dx) in enumerate(offs):
                if oi >= K:
                    d2P = psum.tile([NB, 2, 512], F32, name="d2P", tag="d2P",
                                    bufs=2)
                    kgt = pool.tile([NB, TR, CB], F16, name="kgt", tag="kgt",
                                    bufs=3)
                    sbq = [pool.tile([NB, TR, CB], F16, name=f"sbq{c}",
                                     tag=f"sbq{c}", bufs=2) for c in range(3)]
                else:
                    d2P, kgt = None, heads[oi]
                stt = pool.tile([NB, TR, CB], F16, name="stt", tag="stt")
                trm = pool.tile([NB, TR, CB], F16, name="trm", tag="trm",
                                bufs=2)
                npr = pool.tile([NB, 3, TR, CB], F16, name="nprA",
                                tag="nprA", bufs=2)
                if pend is not None:
                    finish(pend)
                nc.vector.tensor_mul(
                    npr[:], npn[:, :, 2 + dy:90 + dy, 2 + dx:12 + dx],
                    ngn[:])
                if oi >= K:
                    emit_d2(oi, dy, dx, d2P, kgt, sbq)
                if pend is not None:
                    acc_trm(pend[3], pend[4])
                for c in range(3):
                    for ch in range(2):
                        rs = slice(ch * HH, (ch + 1) * HH)
                        nc.tensor.matmul(ndP[:, ch, 0:HH * CB]
                                         .rearrange("p (r c) -> p r c", c=CB),
                                         idt[:], npr[:, c, rs, :],
                                         start=(c == 0), stop=(c == 2))
                if pend is not None:
                    acc_kgt(pend[1], pend[4])
                pend = (d2P, kgt, stt, trm, oi)
            finish(pend)
            acc_trm(pend[3], pend[4])
            acc_kgt(pend[1], pend[4])

            # ---------------- masked reduction ----------------
            nc.vector.tensor_mul(mkt[:, 0:HH, :], accP[0][:], mkt[:, 0:HH, :])
            nc.vector.tensor_mul(mkt[:, HH:TR, :], accP[1][:], mkt[:, HH:TR, :])
            red = pool.tile([NB, 1], F32, name="red", tag="idt01x")
            nc.vector.tensor_reduce(red[:], mkt[:], mybir.AxisListType.XY,
                                    ALU.add)
            nc.sync.dma_start(out=out_d[0:NB, t:t + 1], in_=red[:])

    nc.compile()
    return nc


def _consts():
    idm = np.eye(NB, dtype=np.float16)
    idm01 = (0.1 * np.eye(NB)).astype(np.float16)
    return idm, idm01


def _blockify(slab, r00, nrows, c0=0, ncols=BW):
    """[rows, SW] slab -> [NT, NB, nrows, ncols] block layout."""
    out = np.empty((NT, NB, nrows, ncols), dtype=np.float32)
    cols = (CB * np.arange(NB)[:, None] + c0 + np.arange(ncols)[None, :])
    for t in range(NT):
        x = slab[r00 + t * TR: r00 + t * TR + nrows]     # [nrows, SW]
        out[t] = x[:, cols].transpose(1, 0, 2)
    return out


def _strips(xy1_b, dp_b, r0_img):
    """Window-phase xp values (SQS-scaled) for slab rows [0:3) and [179:182)."""
    out = np.zeros((2, 3, NB, 3, BW), dtype=np.float32)
    for side, base in ((0, r0_img - 3), (1, r0_img + SH)):
        vals = np.full((3, 3, SW), PZ, dtype=np.float32)
        for i in range(3):
            y = base + i
            if 0 <= y < H:
                row = np.full((3, SW), PZ, dtype=np.float32)
                row[:, 3:3 + W] = (SQS * xy1_b[:, y, :]) * dp_b[y, :]
                row[:, 1:3] = PZ
                row[:, 3 + W:3 + W + 2] = PZ
                vals[:, i, :] = row
        for p in range(NB):
            out[side, :, p, :, :] = vals[:, :, CB * p:CB * p + BW]
    return out


def kernel(depth_pred, depth_gt, xy1_grid, K, mask):
    if "nc" not in _prog_cache:
        _prog_cache["nc"] = _build_program()
    nc = _prog_cache["nc"]

    dp = np.asarray(depth_pred, dtype=np.float32).reshape(B, H, W)
    dg = np.asarray(depth_gt, dtype=np.float32).reshape(B, H, W)
    xy1 = np.asarray(xy1_grid, dtype=np.float32)
    mk = np.asarray(mask).reshape(B, H, W)

    # factor the z channel into the depth planes: the device computes
    # xyz = (xy1':z=1) * dz with dz = SQS*xy1_z*depth, which equals
    # SQS*xy1*depth exactly when xy1_z==1 (the intrinsics-grid case).
    z = xy1[:, 2]
    if np.all(z == 1.0):
        xy1f = xy1[:, :2]
        dpf, dgf = SQS * dp, SQS * dg
    else:
        zs = np.where(np.abs(z) > 1e-30, z, 1.0)
        xy1f = xy1[:, :2] / zs[:, None]
        dpf, dgf = (SQS * z) * dp, (SQS * z) * dg

    idm, idm01 = _consts()
    pzc = np.full((3, RB, 2), PZ, dtype=np.float32)
    in_maps = []
    for core in range(N_CORES):
        b, half = core // 2, core % 2
        r0 = half * SH
        lo, hi = r0 - 3, r0 + SH + 3
        slo, shi = max(lo, 0), min(hi, H)
        dps = np.zeros((SH + 6, SW), dtype=np.float32)
        dgs = np.zeros((SH + 6, SW), dtype=np.float32)
        xys = np.zeros((2, SH + 6, SW), dtype=np.float32)
        dps[slo - lo:shi - lo, 3:3 + W] = dpf[b, slo:shi]
        dgs[slo - lo:shi - lo, 3:3 + W] = dgf[b, slo:shi]
        xys[:, slo - lo:shi - lo, 3:3 + W] = xy1f[b, :, slo:shi]
        mks = np.zeros((SH, SW), dtype=np.float32)
        mks[:, 3:3 + W] = mk[b, r0:r0 + SH]
        in_maps.append({
            "dp": _blockify(dps, 0, RB),
            "dg": _blockify(dgs, 0, RB),
            "xy1": np.stack([_blockify(xys[c], 0, RB) for c in range(2)]),
            "mk": _blockify(mks, 0, TR, c0=3, ncols=CB),
            "strip": _strips(xy1[b], dp[b], r0),
            "pzc": pzc, "idm": idm, "idm01": idm01,
        })

    res = run_bass_kernel_spmd(nc, in_maps, list(range(N_CORES)))
    total = 0.0
    for core in range(N_CORES):
        total += res.results[core]["out"][0:NB, :].astype(np.float64).sum()
    nval = float(mk.sum(dtype=np.float64))
    return np.float32(-total / (nval + EPS))


# revision 38
# speedup vs baseline: 1.0114x; 1.0114x over previous
"""C3D loss kernel for Trainium2 (8 NeuronCores, Bass/Tile).

Sharding: pure data parallel over B*2 = 8 shards (each image split into
top/bottom 176-row halves). Each core computes a partial sum of the loss
numerator; host combines and divides by the valid count.

Layout: partitions = 122 column blocks of 10 pixels (3+3 col halo -> 16
stored cols per block); free dims = (rows, 16). Every spatial shift (the
5x5 window and the normal central differences) is a free-dim offset, which
keeps all engine accesses at partition start 0 (a hardware requirement).

v3 design:
- Inputs staged on the host directly in block layout [NT, NB, rows, 16],
  so each load DMA moves contiguous 6KB runs per partition.
- The host folds the SQS prescale and the xy1 z-channel into the depth
  planes (xyz_z == scaled depth), so only x/y need on-device muls.
- No channel stacking: window diffs / squares / normal products run
  per-channel on [122, 880] views, and the channel sum d2 = sum_c diff_c^2
  rides the SAME identity-stationary PSUM-accumulating matmuls that the
  offset accumulation already uses. This removes 36 SBUF->SBUF stacking
  DMAs per tile.
- A custom DVE op SQDIFF_C3D computes (a-b)^2 in one f32-in instruction;
  a policy splits offsets between it and Pool-sub+Act-square to balance
  engines.
- |nd| rides Act as Abs(1.9*nd); the +0.1 coefficient is a second
  accumulation matmul with 0.1*I stationary.
- The normal chain stays f32 through the cross product (gx ~parallel~ gy
  amplifies rounding ~30x); gt normals only computed on the 88x10 output
  domain.
- exp/abs/trm for offset o-1 are emitted at the head of turn o and the
  accumulation matmuls one turn later, so Act/DVE/PE pipeline across
  offsets.
"""
import sys

sys.path.insert(0, "/opt/trn_rl_repo")

import numpy as np
from contextlib import ExitStack

import bass_rust
import concourse.bass as bass
import concourse.tile as tile
import concourse.dve_ops as dve_ops
import concourse.dve_spec as dve_spec
from concourse.dve_spec import Spec, Src0, Src1, sq
from concourse.dve_uop import DveOpSpec
from concourse import bacc, mybir
from concourse.bass_utils import run_bass_kernel_spmd

F32 = mybir.dt.float32
F16 = mybir.dt.float16
AF = mybir.ActivationFunctionType
ALU = mybir.AluOpType

B, H, W = 4, 352, 1216
R = 2
ELL = 0.05
INV2ELL2 = float(np.float32(1.0 / (2.0 * ELL * ELL)))   # 200.0
EPS = 1e-8
N_CORES = 8

SH = H // 2          # shard rows per core = 176
NT = 2               # row tiles per core
TR = SH // NT        # output rows per tile = 88
HH = TR // 2         # PSUM chunk rows = 44
RB = TR + 6          # stored rows per tile = 94
CB = 10              # cols per block
NB = 122             # blocks
BW = CB + 6          # stored cols per block = 16
SW = CB * (NB - 1) + BW   # slab width = 1226 (slab col j <-> image col j-3)
SQS = 0.0625         # pre-scale (2^-4, exact) folded into inputs on host
PZ = 2000.0 * SQS    # poison depth in scaled units = 125
EXS = float(INV2ELL2 / (SQS * SQS))    # exp scale compensation = 51200
LN14 = float(np.log(0.25))

# engine policy for the per-offset squared diffs, indexed by
# (t*25+oi)*3+c mod len: 'a' = fused SQDIFF on DVE,
# 'b' = sub on Pool + square on Act, 'c' = sub on Pool + square on DVE
POL_SBSQ = ('a', 'b', 'b', 'a', 'b', 'b', 'a', 'b', 'a', 'b')
NPR_POOL = 9999  # every NPR_POOL-th npr mul runs on Pool instead of DVE
NLD = 4       # DMA chunks per input plane load
_prog_cache = {}


def _register_sqdiff():
    name = "SQDIFF_C3D"
    if name in dve_ops._SUB_OPCODE_FOR_NAME:
        for o in dve_ops.OPS:
            if o.name == name:
                return o
    spec = Spec(
        body=sq(Src0 - Src1),
        reference=lambda in0, in1, s0, s1, imm2:
            ((in0.astype(np.float32) - in1) ** 2).astype(np.float32))
    row = max(dve_ops._SUB_OPCODE_FOR_NAME.values()) + 1
    assert row < 0x20
    dve_ops._SUB_OPCODE_FOR_NAME[name] = row
    shas = {}
    for ver in ("v3", "v4"):
        uops = dve_spec.lower(spec, ver=ver)
        s = DveOpSpec(name=name, opcode=row, uops=uops,
                      rd1_en=dve_spec._has_src1(spec))
        shas[ver] = s.sha(ver)
    op = dve_ops.DveOp(name, spec, subdim=False, uops_sha=shas)
    dve_ops.OPS.append(op)
    dve_ops.CUSTOM_DVE_SPECS[name] = spec
    return op


def _build_program():
    sqdiff = _register_sqdiff()
    nc = bacc.Bacc("TRN2", target_bir_lowering=False, debug=False,
                   num_devices=N_CORES)

    for v in (EPS, LN14, 1e-30):
        t = nc.alloc_sbuf_tensor(f"const-f32-{v}", [128, 1], F32)
        nc.gpsimd.memset(t.ap(), v)
        nc.const_aps.aps[(F32, v)] = t.ap()
    nc.all_engine_barrier()

    dp_d = nc.dram_tensor("dp", [NT, NB, RB, BW], F32, kind="ExternalInput").ap()
    dg_d = nc.dram_tensor("dg", [NT, NB, RB, BW], F32, kind="ExternalInput").ap()
    xy1_d = nc.dram_tensor("xy1", [2, NT, NB, RB, BW], F32,
                           kind="ExternalInput").ap()
    mk_d = nc.dram_tensor("mk", [NT, NB, TR, CB], F32, kind="ExternalInput").ap()
    st_d = nc.dram_tensor("strip", [2, 3, NB, 3, BW], F32,
                          kind="ExternalInput").ap()
    pz_d = nc.dram_tensor("pzc", [3, RB, 2], F32, kind="ExternalInput").ap()
    id_d = nc.dram_tensor("idm", [NB, NB], F16, kind="ExternalInput").ap()
    id01_d = nc.dram_tensor("idm01", [NB, NB], F16, kind="ExternalInput").ap()
    out_d = nc.dram_tensor("out", [128, NT], F32, kind="ExternalOutput").ap()

    def chunked_load(dst, src):
        step = (NB + NLD - 1) // NLD
        for i0 in range(0, NB, step):
            i1 = min(i0 + step, NB)
            nc.sync.dma_start(out=dst[i0:i1], in_=src[i0:i1])

    with tile.TileContext(nc) as tc, ExitStack() as ctx:
        pool = ctx.enter_context(tc.tile_pool(name="p", bufs=1))
        psum = ctx.enter_context(tc.tile_pool(name="ps", bufs=1, space="PSUM"))
        idt = pool.tile([NB, NB], F16, name="idt")
        nc.sync.dma_start(out=idt[:], in_=id_d[:])
        idt01 = pool.tile([NB, NB], F16, name="idt01")
        nc.sync.dma_start(out=idt01[:], in_=id01_d[:])

        for t in range(NT):
            # ---------------- input loads (block layout, contiguous) -------
            dpt = pool.tile([NB, RB, BW], F32, name="dpt", bufs=2)
            chunked_load(dpt, dp_d[t])
            dgt = pool.tile([NB, RB, BW], F32, name="dgt", bufs=2)
            chunked_load(dgt, dg_d[t])
            xy1t = [pool.tile([NB, RB, BW], F32, name=f"xy1t{c}",
                              bufs=(2 if c == 0 else 1))
                    for c in range(2)]
            for c in range(2):
                chunked_load(xy1t[c], xy1_d[c, t])
            mkt = pool.tile([NB, TR, CB], F32, name="mkt")
            nc.sync.dma_start(out=mkt[:], in_=mk_d[t])

            # ------- xyz: z plane IS the (scaled) depth plane -------
            xp = [pool.tile([NB, RB, BW], F32, name=f"xp{c}", bufs=2)
                  for c in range(2)]
            xg = [pool.tile([NB, RB, BW], F32, name=f"xg{c}", bufs=2)
                  for c in range(2)]
            for c in range(2):
                nc.vector.tensor_mul(xp[c][:], xy1t[c][:], dpt[:])
                nc.gpsimd.tensor_mul(xg[c][:, 2:92, 2:14], xy1t[c][:, 2:92, 2:14],
                                     dgt[:, 2:92, 2:14])
            xp.append(dpt)
            xg.append(dgt)

            # ---------------- normals (f32 chain) ----------------
            def w3(x, dr, dc):
                return x[:, 1 + dr:93 + dr, 1 + dc:15 + dc]

            nrm = {}
            for key, xc in (("p", xp), ("g", xg)):
                seng = nc.vector if key == "p" else nc.gpsimd
                gx = [pool.tile([NB, 92, 14], F32, name=f"gx{c}")
                      for c in range(3)]
                gy = [pool.tile([NB, 92, 14], F32, name=f"gy{c}")
                      for c in range(3)]
                if key == "p":
                    sub = lambda x: x[:, 0:92, 0:14]
                    vx0, vx1 = (lambda c: w3(xc[c], 0, 1)), (lambda c: w3(xc[c], 0, -1))
                    vy0, vy1 = (lambda c: w3(xc[c], 1, 0)), (lambda c: w3(xc[c], -1, 0))
                else:
                    sub = lambda x: x[:, 0:TR, 0:CB]
                    vx0 = lambda c: xc[c][:, 3:91, 4:14]
                    vx1 = lambda c: xc[c][:, 3:91, 2:12]
                    vy0 = lambda c: xc[c][:, 4:92, 3:13]
                    vy1 = lambda c: xc[c][:, 2:90, 3:13]
                for c in range(3):
                    seng.tensor_sub(sub(gx[c]), vx0(c), vx1(c))
                    nc.vector.tensor_sub(sub(gy[c]), vy0(c), vy1(c))
                cr = [pool.tile([NB, 92, 14], F32, name=f"cr{c}")
                      for c in range(3)]
                tA = pool.tile([NB, 92, 14], F32, name="tA")
                for c in range(3):
                    a, b = (c + 1) % 3, (c + 2) % 3
                    nc.vector.tensor_mul(sub(cr[c]), sub(gx[a]), sub(gy[b]))
                    seng.tensor_mul(sub(tA), sub(gx[b]), sub(gy[a]))
                    seng.tensor_sub(sub(cr[c]), sub(cr[c]), sub(tA))
                q = pool.tile([NB, 92, 14], F32, name="q")
                sqt = pool.tile([NB, 92, 14], F32, name="sqt", tag="tA")
                nc.scalar.activation(sub(q), sub(cr[0]), AF.Square)
                nc.scalar.activation(sub(sqt), sub(cr[1]), AF.Square)
                seng.tensor_add(sub(q), sub(q), sub(sqt))
                nc.scalar.activation(sub(sqt), sub(cr[2]), AF.Square)
                seng.tensor_add(sub(q), sub(q), sub(sqt))
                # w = q^-0.5 = exp(-0.5*ln(q+1e-30)); exact-zero borders
                # give w=1e15 * cr=0 -> n=0, matching n/(|n|+eps)
                nc.scalar.activation(sub(q), sub(q), AF.Ln, bias=1e-30)
                nc.scalar.activation(sub(q), sub(q), AF.Exp, scale=-0.5)
                nshape = [NB, 3, 92, 14] if key == "p" else [NB, 3, TR, CB]
                nt_ = pool.tile(nshape, F16, name=f"n{key}A", bufs=2)
                for c in range(3):
                    nc.vector.tensor_mul(sub(nt_[:, c]), sub(cr[c]), sub(q))
                nrm[key] = nt_
            npn, ngn = nrm["p"], nrm["g"]

            # ------- poison xp at out-of-image rows/cols (after normals) ---
            for c in range(3):
                if t == 0:
                    nc.sync.dma_start(out=xp[c][:, 1:3, 1:15],
                                      in_=st_d[0, c, :, 1:3, 1:15])
                if t == NT - 1:
                    nc.sync.dma_start(out=xp[c][:, 91:93, 1:15],
                                      in_=st_d[1, c, :, 0:2, 1:15])
                nc.sync.dma_start(out=xp[c][0:1, 1:93, 1:3],
                                  in_=pz_d[c, 0:92, :])
                nc.sync.dma_start(out=xp[c][NB - 1:NB, 1:93, 9:11],
                                  in_=pz_d[c, 0:92, :])

            # ---------------- window phase ----------------
            accP = [psum.tile([NB, HH, CB], F32, name=f"accP{ch}") for ch in range(2)]
            ndP = psum.tile([NB, 2, 512], F32, name="ndP")

            noff = (2 * R + 1) ** 2
            offs = [(dy, dx) for dy in range(-R, R + 1) for dx in range(-R, R + 1)]

            def acc_trm(ptrm, poi):
                for ch in range(2):
                    rs = slice(ch * HH, (ch + 1) * HH)
                    nc.tensor.matmul(accP[ch][:], idt[:], ptrm[:, rs, :],
                                     start=(poi == 0), stop=False)

            def acc_kgt(pkgt, poi):
                for ch in range(2):
                    rs = slice(ch * HH, (ch + 1) * HH)
                    nc.tensor.matmul(accP[ch][:], idt01[:], pkgt[:, rs, :],
                                     start=False, stop=(poi == noff - 1))

            def finish(pend):
                # exp/abs/trm for the PREVIOUS offset: at the head of the
                # next turn so Act/DVE don't lock-step with the PE.
                d2p, kgt, stt, trm, poi = pend
                if d2p is not None:
                    nc.scalar.activation(
                        kgt[:].rearrange("p (a r) c -> p a (r c)", a=2),
                        d2p[:, :, 0:HH * CB], AF.Exp, scale=-EXS)
                nc.scalar.activation(
                    stt[:].rearrange("p (a r) c -> p a (r c)", a=2),
                    ndP[:, :, 0:HH * CB], AF.Abs, scale=1.9)
                nc.vector.tensor_mul(trm[:], stt[:], kgt[:])

            def emit_d2(oi, dy, dx, d2P, kgt, sbq):
                for c in range(3):
                    xsv = xp[c][:, 3 + dy:91 + dy, 3 + dx:13 + dx]
                    xgv = xg[c][:, 3:91, 3:13]
                    gi = (t * noff + oi) * 3 + c
                    var = POL_SBSQ[gi % len(POL_SBSQ)]
                    if var == 'a':
                        nc.vector._custom_dve(sqdiff, out=sbq[c][:],
                                              in0=xsv, in1=xgv)
                    else:
                        nc.gpsimd.tensor_sub(sbq[c][:], xsv, xgv)
                        if var == 'b':
                            nc.scalar.activation(sbq[c][:], sbq[c][:],
                                                 AF.Square)
                        else:
                            nc.vector.tensor_mul(sbq[c][:], sbq[c][:],
                                                 sbq[c][:])
                for c in range(3):
                    for ch in range(2):
                        rs = slice(ch * HH, (ch + 1) * HH)
                        nc.tensor.matmul(d2P[:, ch, 0:HH * CB]
                                         .rearrange("p (r c) -> p r c", c=CB),
                                         idt[:], sbq[c][:, rs, :],
                                         start=(c == 0), stop=(c == 2))

            # head start: the d2 side of the first K offsets (+ eager exp)
            # only needs xp/xg/poison, so it runs while the normal chain
            # finishes; the nd side catches up in the main loop.
            K = 2 if t == 0 else 0
            heads = {}
            for oi in range(K):
                dy, dx = offs[oi]
                d2P = psum.tile([NB, 2, 512], F32, name="d2P", tag="d2P",
                                bufs=2)
                kgt = pool.tile([NB, TR, CB], F16, name="kgt", tag="kgt",
                                bufs=3)
                sbq = [pool.tile([NB, TR, CB], F16, name=f"sbq{c}",
                                 tag=f"sbq{c}", bufs=2) for c in range(3)]
                emit_d2(oi, dy, dx, d2P, kgt, sbq)
                nc.scalar.activation(
                    kgt[:].rearrange("p (a r) c -> p a (r c)", a=2),
                    d2P[:, :, 0:HH * CB], AF.Exp, scale=-EXS)
                heads[oi] = kgt

            pend = None
            for oi, (dy, dx) in enumerate(offs):
                if oi >= K:
                    d2P = psum.tile([NB, 2, 512], F32, name="d2P", tag="d2P",
                                    bufs=2)
                    kgt = pool.tile([NB, TR, CB], F16, name="kgt", tag="kgt",
                                    bufs=3)
                    sbq = [pool.tile([NB, TR, CB], F16, name=f"sbq{c}",
                                     tag=f"sbq{c}", bufs=2) for c in range(3)]
                else:
                    d2P, kgt = None, heads[oi]
                stt = pool.tile([NB, TR, CB], F16, name="stt", tag="stt")
                trm = pool.tile([NB, TR, CB], F16, name="trm", tag="trm",
                                bufs=2)
                npr = pool.tile([NB, 3, TR, CB], F16, name="nprA",
                                tag="nprA", bufs=2)
                if pend is not None:
                    finish(pend)
                nc.vector.tensor_mul(
                    npr[:], npn[:, :, 2 + dy:90 + dy, 2 + dx:12 + dx],
                    ngn[:])
                if oi >= K:
                    emit_d2(oi, dy, dx, d2P, kgt, sbq)
                if pend is not None:
                    acc_trm(pend[3], pend[4])
                for c in range(3):
                    for ch in range(2):
                        rs = slice(ch * HH, (ch + 1) * HH)
                        nc.tensor.matmul(ndP[:, ch, 0:HH * CB]
                                         .rearrange("p (r c) -> p r c", c=CB),
                                         idt[:], npr[:, c, rs, :],
                                         start=(c == 0), stop=(c == 2))
                if pend is not None:
                    acc_kgt(pend[1], pend[4])
                pend = (d2P, kgt, stt, trm, oi)
            finish(pend)
            acc_trm(pend[3], pend[4])
            acc_kgt(pend[1], pend[4])

            # ---------------- masked reduction ----------------
            nc.vector.tensor_mul(mkt[:, 0:HH, :], accP[0][:], mkt[:, 0:HH, :])
            nc.vector.tensor_mul(mkt[:, HH:TR, :], accP[1][:], mkt[:, HH:TR, :])
            red = pool.tile([NB, 1], F32, name="red", tag="idt01x")
            nc.vector.tensor_reduce(red[:], mkt[:], mybir.AxisListType.XY,
                                    ALU.add)
            nc.sync.dma_start(out=out_d[0:NB, t:t + 1], in_=red[:])

    nc.compile()
    return nc


def _consts():
    idm = np.eye(NB, dtype=np.float16)
    idm01 = (0.1 * np.eye(NB)).astype(np.float16)
    return idm, idm01


def _blockify(slab, r00, nrows, c0=0, ncols=BW):
    """[rows, SW] slab -> [NT, NB, nrows, ncols] block layout."""
    out = np.empty((NT, NB, nrows, ncols), dtype=np.float32)
    cols = (CB * np.arange(NB)[:, None] + c0 + np.arange(ncols)[None, :])
    for t in range(NT):
        x = slab[r00 + t * TR: r00 + t * TR + nrows]     # [nrows, SW]
        out[t] = x[:, cols].transpose(1, 0, 2)
    return out


def _strips(xy1_b, dp_b, r0_img):
    """Window-phase xp values (SQS-scaled) for slab rows [0:3) and [179:182)."""
    out = np.zeros((2, 3, NB, 3, BW), dtype=np.float32)
    for side, base in ((0, r0_img - 3), (1, r0_img + SH)):
        vals = np.full((3, 3, SW), PZ, dtype=np.float32)
        for i in range(3):
            y = base + i
            if 0 <= y < H:
                row = np.full((3, SW), PZ, dtype=np.float32)
                row[:, 3:3 + W] = (SQS * xy1_b[:, y, :]) * dp_b[y, :]
                row[:, 1:3] = PZ
                row[:, 3 + W:3 + W + 2] = PZ
                vals[:, i, :] = row
        for p in range(NB):
            out[side, :, p, :, :] = vals[:, :, CB * p:CB * p + BW]
    return out


def kernel(depth_pred, depth_gt, xy1_grid, K, mask):
    if "nc" not in _prog_cache:
        _prog_cache["nc"] = _build_program()
    nc = _prog_cache["nc"]

    dp = np.asarray(depth_pred, dtype=np.float32).reshape(B, H, W)
    dg = np.asarray(depth_gt, dtype=np.float32).reshape(B, H, W)
    xy1 = np.asarray(xy1_grid, dtype=np.float32)
    mk = np.asarray(mask).reshape(B, H, W)

    # factor the z channel into the depth planes: the device computes
    # xyz = (xy1':z=1) * dz with dz = SQS*xy1_z*depth, which equals
    # SQS*xy1*depth exactly when xy1_z==1 (the intrinsics-grid case).
    z = xy1[:, 2]
    if np.all(z == 1.0):
        xy1f = xy1[:, :2]
        dpf, dgf = SQS * dp, SQS * dg
    else:
        zs = np.where(np.abs(z) > 1e-30, z, 1.0)
        xy1f = xy1[:, :2] / zs[:, None]
        dpf, dgf = (SQS * z) * dp, (SQS * z) * dg

    idm, idm01 = _consts()
    pzc = np.full((3, RB, 2), PZ, dtype=np.float32)
    in_maps = []
    for core in range(N_CORES):
        b, half = core // 2, core % 2
        r0 = half * SH
        lo, hi = r0 - 3, r0 + SH + 3
        slo, shi = max(lo, 0), min(hi, H)
        dps = np.zeros((SH + 6, SW), dtype=np.float32)
        dgs = np.zeros((SH + 6, SW), dtype=np.float32)
        xys = np.zeros((2, SH + 6, SW), dtype=np.float32)
        dps[slo - lo:shi - lo, 3:3 + W] = dpf[b, slo:shi]
        dgs[slo - lo:shi - lo, 3:3 + W] = dgf[b, slo:shi]
        xys[:, slo - lo:shi - lo, 3:3 + W] = xy1f[b, :, slo:shi]
        mks = np.zeros((SH, SW), dtype=np.float32)
        mks[:, 3:3 + W] = mk[b, r0:r0 + SH]
        in_maps.append({
            "dp": _blockify(dps, 0, RB),
            "dg": _blockify(dgs, 0, RB),
            "xy1": np.stack([_blockify(xys[c], 0, RB) for c in range(2)]),
            "mk": _blockify(mks, 0, TR, c0=3, ncols=CB),
            "strip": _strips(xy1[b], dp[b], r0),
            "pzc": pzc, "idm": idm, "idm01": idm01,
        })

    res = run_bass_kernel_spmd(nc, in_maps, list(range(N_CORES)))
    total = 0.0
    for core in range(N_CORES):
        total += res.results[core]["out"][0:NB, :].astype(np.float64).sum()
    nval = float(mk.sum(dtype=np.float64))
    return np.float32(-total / (nval + EPS))
